# revision 47
# baseline (speedup 1.0000x reference)
"""DiT block kernel for 8 Trainium2 NeuronCores — fp8 DoubleRow redesign.

Sharding: data-parallel over (batch, seq-half) -> 8 shards, no collectives.
Each core gets x[b] rotated so its 512 query tokens are rows 0:511; K/V are
computed over the full (rotated) 1024-token sequence.

Key ideas vs the fp32r baseline:
- The adaLN modulation (a [1,128]@[128,6144] matmul) is computed on the HOST
  inside kernel(); (1+scale)*norm_w folds into weight rows, shifts into bias
  rows, gates into w_out/w2 columns. The device kernel runs plain LayerNorm.
- All large matmuls run as fp8e4m3 DoubleRow (2 k-tiles of 128 per
  instruction at 0.5 cycles/row): qkv, scores, PV, out-proj, fc1, fc2.
  Weights are host-quantized with power-of-2 scales (descaled for free in
  the fused psum-export ops / gelu scale).
- Transposes go through the XBAR DMA engine (dma_start_transpose, bf16) on
  the SP queue instead of the PE array; rope runs after the transpose in a
  head-permuted column layout (partner halves land in adjacent c-slots, so
  rope is full-128-partition work on Pool and writes fp8 directly).
- LayerNorm stats via bn_stats/bn_aggr on DVE; normalize+cast on Pool.
"""

import sys

sys.path.insert(0, "/opt/trn_rl_repo")

import numpy as np
import ml_dtypes

import concourse.bass as bass
import concourse.mybir as mybir
from concourse.bass_utils import run_bass_kernel_spmd
from concourse.tile import TileContext
from concourse.vector_clock import ScopedClock, VectorClock

# ---------------------------------------------------------------------------
# Walrus in this container caps sync-wait commands per CTRL instruction at a
# small number; Tile's stock tail drain collects one wait per live proc and
# trips that cap. Split the final waits across one SP NOP per proc instead.
_orig_drain_and_barrier = TileContext._drain_and_barrier


def _split_drain_and_barrier(self, tick_clock, wait_clock):
    gc_list = list(tick_clock.global_clock)
    for p, tick in enumerate(gc_list):
        if tick > 0:
            partial = [0] * len(gc_list)
            partial[p] = tick
            nop = self.nc.sync.nop()
            wait_clock.add_sem_waits(nop.ins, ScopedClock({None: VectorClock(partial)}))
    drain_inst = self.nc.sync.drain()
    req = ScopedClock({None: tick_clock.global_clock.copy()})
    cur = ScopedClock({None: tick_clock.global_clock.copy()})
    wait_clock.add_sem_waits(drain_inst.ins, req, cur)
    self.nc.all_engine_barrier()
    popped = self.nc._tile_sem_poison_stack.pop()
    assert popped is self._sem_poison
    self.nc.clear_and_free_semaphores(list(self.sems.allocated().values()))
    self.nc.all_engine_barrier()


TileContext._drain_and_barrier = _split_drain_and_barrier

# This walrus also caps waits per *compute/DMA* instruction (the S3_LW struct
# allows a single wait). Intercept every instruction Tile commits to a basic
# block and spill all but the last wait onto preceding same-engine NOPs.
_nop_proto = None


def _get_nop_proto():
    global _nop_proto
    if _nop_proto is None:
        scratch = bass.Bass()
        _nop_proto = scratch.sync.nop().ins
        _nop_proto.sync_info = None
    return _nop_proto


_orig_add_instruction = TileContext._add_instruction


def _add_instruction_capped(self, inst):
    si = inst.sync_info
    if si is not None and si.on_wait is not None and len(si.on_wait) > 1:
        waits = list(si.on_wait)
        si.on_wait = waits[-1:]
        import copy as _copy

        for w in waits[:-1]:
            nop = _copy.deepcopy(_get_nop_proto())
            nop.name = self.nc.get_next_instruction_name()
            nop.engine = inst.engine
            nop.sync_info = mybir.SyncInfo(on_wait=[w], on_update=[])
            _orig_add_instruction(self, nop)
    _orig_add_instruction(self, inst)


TileContext._add_instruction = _add_instruction_capped

# Capture the Tile scheduler's modeled makespan (cost-model ns) per block —
# the only timing signal available in this container (no NTFF profiling).
MODELED_NS = []
from concourse.bass_interp import CoreSim as _CoreSim

_orig_coresim_simulate = _CoreSim.simulate


def _simulate_capture(self, *a, **k):
    r = _orig_coresim_simulate(self, *a, **k)
    try:
        if self.is_scheduling_pass():
            MODELED_NS.append((getattr(self, "name", "?"), int(self.time)))
    except Exception:
        pass
    return r


_CoreSim.simulate = _simulate_capture


def _publish_perfetto_local(self):
    """Dump the scheduling-sim perfetto trace to a local file (no upload)."""
    import os as _os

    if self.perfetto is None:
        return
    path = _os.environ.get("BASS_MODEL_TRACE_PATH", "/tmp/model_trace.pftrace")
    with open(path, "wb") as f:
        f.write(bytes(self.perfetto.take_serialized()))
    print(f"[kernel] modeled trace written to {path}")


_CoreSim.publish_perfetto = _publish_perfetto_local

# ---------------------------------------------------------------------------

FP32 = mybir.dt.float32
FP32R = mybir.dt.float32r
FP8 = mybir.dt.float8e4
BF16 = mybir.dt.bfloat16
AF = mybir.ActivationFunctionType
ALU = mybir.AluOpType
PM = mybir.MatmulPerfMode

D = 1024
H = 16
HD = 64
S = 1024
SQ = 512  # query tokens per core
MLP = 4096
NT = S // 128  # 8 token tiles
NTQ = SQ // 128  # 4 query token tiles
LN_EPS = 1e-5
N_CORES = 8

NP_FP8 = ml_dtypes.float8_e4m3
NP_BF16 = ml_dtypes.bfloat16


def _build_nc(reps=1):
    import os as _os

    trace_sim = bool(_os.environ.get("BASS_MODEL_TRACE"))
    nc = bass.Bass()

    xb = nc.dram_tensor("xb", [S, D], BF16, kind="ExternalInput")
    wq = nc.dram_tensor("wq", [128, 4, 2, D], FP8, kind="ExternalInput")
    wk = nc.dram_tensor("wk", [128, 4, 2, D], FP8, kind="ExternalInput")
    wv = nc.dram_tensor("wv", [128, 4, 2, D], FP8, kind="ExternalInput")
    wo = nc.dram_tensor("wo", [128, 4, 2, D], FP8, kind="ExternalInput")
    w1 = nc.dram_tensor("w1", [128, 8, 2, MLP], FP8, kind="ExternalInput")
    w2 = nc.dram_tensor("w2", [128, 32, 2, D], FP8, kind="ExternalInput")
    cosT = nc.dram_tensor("cosT", [128, S], BF16, kind="ExternalInput")
    sinT = nc.dram_tensor("sinT", [128, S], BF16, kind="ExternalInput")
    bq = nc.dram_tensor("bq", [128, D], BF16, kind="ExternalInput")
    bk = nc.dram_tensor("bk", [128, D], BF16, kind="ExternalInput")
    bv = nc.dram_tensor("bv", [128, D], BF16, kind="ExternalInput")
    b2r = nc.dram_tensor("b2r", [128, D], BF16, kind="ExternalInput")
    b1r = nc.dram_tensor("b1r", [128, 32], FP32, kind="ExternalInput")
    scl = nc.dram_tensor("scl", [128, 8], FP32, kind="ExternalInput")
    outd = nc.dram_tensor("out", [SQ, D], FP32, kind="ExternalOutput")

    with TileContext(nc, trace_sim=trace_sim) as tc:
        for _rep in range(reps):
            _emit_body(nc, tc, xb, wq, wk, wv, wo, w1, w2, cosT, sinT,
                       bq, bk, bv, b2r, b1r, scl, outd)
    return nc


def _emit_body(nc, tc, xb, wq, wk, wv, wo, w1, w2, cosT, sinT,
               bq, bk, bv, b2r, b1r, scl, outd):
    # ------------------------------------------------------------ persistent
    const_cm = tc.tile_pool(name="const", bufs=1)
    const = const_cm.__enter__()
    epst = const.tile([128, 1], FP32, tag="eps")
    nc.vector.memset(epst, LN_EPS)
    ones64f = const.tile([1, 64], FP32, tag="o64f")
    nc.vector.memset(ones64f, 1.0)
    ones64 = const.tile([1, 64], FP32R, tag="o64")
    nc.vector.tensor_copy(ones64, ones64f)

    rows_cm = tc.tile_pool(name="rows", bufs=1)
    rows = rows_cm.__enter__()
    scl_sb = rows.tile([128, 8], FP32, tag="scl")
    nc.scalar.dma_start(out=scl_sb, in_=scl[:, :])
    bqb = rows.tile([128, D], BF16, tag="bqb")
    nc.scalar.dma_start(out=bqb, in_=bq[:, :])
    bkb = rows.tile([128, D], BF16, tag="bkb")
    nc.scalar.dma_start(out=bkb, in_=bk[:, :])
    cosb = rows.tile([128, S], BF16, tag="cosb")
    sinb = rows.tile([128, S], BF16, tag="sinb")
    bvb = rows.tile([128, D], BF16, tag="bvb")
    b2rb = rows.tile([128, D], BF16, tag="b2rb")
    b1c_sb = rows.tile([128, 32], FP32, tag="b1c")

    # x tiles (kept resident: tiles 0-3 are reused as the residual in s4)
    x_cm = tc.tile_pool(name="xp", bufs=1)
    xp = x_cm.__enter__()
    xt = xp.tile([128, NT, D], BF16, tag="xt")
    for tt in range(NT):
        nc.sync.dma_start(out=xt[:, tt, :], in_=xb[tt * 128 : (tt + 1) * 128, :])

    # long-lived pools opened early so pool lifetimes nest (LIFO releases)
    x1_cm = tc.tile_pool(name="x1p", bufs=1)
    x1p = x1_cm.__enter__()
    x1 = x1p.tile([128, NTQ, D], FP32, tag="x1")
    w2a_cm = tc.tile_pool(name="w2ap", bufs=1)
    w2ap = w2a_cm.__enter__()
    w2a_sb = w2ap.tile([128, 32, 2, 512], FP8, tag="w2a")
    wo_cm = tc.tile_pool(name="wop", bufs=1)
    wop = wo_cm.__enter__()
    wo_sb = wop.tile([128, 4, 2, D], FP8, tag="wo")
    attnT_cm = tc.tile_pool(name="attnTp", bufs=1)
    attnTp = attnT_cm.__enter__()
    attnT8 = attnTp.tile([128, 8, SQ], FP8, tag="attnT8")

    qk_cm = tc.tile_pool(name="qkp", bufs=1)
    qkp = qk_cm.__enter__()
    qTb = qkp.tile([128, 8, SQ], BF16, tag="qTb")
    kTb = qkp.tile([128, 8, S], BF16, tag="kTb")
    qT8 = qkp.tile([128, 8, SQ], FP8, tag="qT8")
    kT8 = qkp.tile([128, 8, S], FP8, tag="kT8")
    v8 = qkp.tile([128, 4, 2, H, HD + 1], FP8, tag="v8")
    nc.gpsimd.memset(v8[:, :, :, :, HD : HD + 1], 1.0)

    # transposed-activation pool (released after V)
    xmT_cm = tc.tile_pool(name="xmTp", bufs=1)
    xmTp = xmT_cm.__enter__()
    xmT8 = xmTp.tile([128, 8, S], FP8, tag="xmT8")

    # q/k/v weights (one pool, released after V)
    wqkv_cm = tc.tile_pool(name="wqkvp", bufs=1)
    wqkvp = wqkv_cm.__enter__()
    wq_sb = wqkvp.tile([128, 4, 2, D], FP8, tag="wq")
    nc.scalar.dma_start(out=wq_sb, in_=wq[:, :, :, :])
    wk_sb = wqkvp.tile([128, 4, 2, D], FP8, tag="wk")
    nc.scalar.dma_start(out=wk_sb, in_=wk[:, :, :, :])
    nc.scalar.dma_start(out=cosb, in_=cosT[:, :])
    nc.scalar.dma_start(out=sinb, in_=sinT[:, :])

    # ------------------------------------------------------------ LN helper
    def ln_tile(src_ap, dst_bf_ap, st, veng=nc.gpsimd):
        """LayerNorm(src) -> dst (bf16). src/dst [128, 1024] APs."""
        st6 = st.tile([128, 2, 6], FP32, tag="st6")
        srcv = src_ap.rearrange("p (a b) -> p a b", a=2)
        nc.vector.bn_stats(st6[:, 0, :], srcv[:, 0, :])
        nc.vector.bn_stats(st6[:, 1, :], srcv[:, 1, :])
        agg = st.tile([128, 2], FP32, tag="agg")
        nc.vector.bn_aggr(agg, st6)
        std = st.tile([128, 1], FP32, tag="std")
        nc.scalar.activation(out=std, in_=agg[:, 1:2], func=AF.Sqrt, bias=epst)
        rstd = st.tile([128, 1], FP32, tag="rstd")
        nc.vector.reciprocal(rstd, std)
        veng.tensor_scalar(
            out=dst_bf_ap, in0=src_ap, scalar1=agg[:, 0:1], scalar2=rstd,
            op0=ALU.subtract, op1=ALU.mult,
        )

    # ------------------------------------------------------ stage A: LN1
    with tc.tile_pool(name="s1st", bufs=3) as s1st, \
         tc.tile_pool(name="s1xm", bufs=2) as s1xm, \
         tc.tile_pool(name="s1tb", bufs=3) as s1tb:
        for tt in range(NT):
            xm = s1xm.tile([128, D], BF16, tag="xm")
            ln_tile(xt[:, tt, :], xm, s1st)
            xmTb = s1tb.tile([128, 8, 128], BF16, tag="xmTb")
            nc.sync.dma_start_transpose(xmTb, xm)
            nc.vector.tensor_copy(
                xmT8[:, :, tt * 128 : (tt + 1) * 128], xmTb
            )

    # ------------------------------------------------------ stage B: QKV
    s2ps_cm = tc.tile_pool(name="s2ps", bufs=4, space="PSUM")
    s2ps = s2ps_cm.__enter__()
    s2sb_cm = tc.tile_pool(name="s2sb", bufs=3)
    s2sb = s2sb_cm.__enter__()
    rsc_cm = tc.tile_pool(name="rsc", bufs=2)
    rsc = rsc_cm.__enter__()

    def qkv_mm(w_sb, tt, fc, nt_tok):
        """One [128tok, 512] qkv psum tile via 4 DoubleRow matmuls."""
        pt = s2ps.tile([128, 512], FP32, tag="qkvp", name="qkvp")
        for j in range(4):
            nc.tensor.matmul(
                pt,
                xmT8[:, 2 * j : 2 * j + 2, tt * 128 : (tt + 1) * 128],
                w_sb[:, j, :, fc * 512 : (fc + 1) * 512],
                start=(j == 0), stop=(j == 3),
                perf_mode=PM.DoubleRow,
            )
        return pt

    def rope_unit(src_bf, dst8, c0, T, tok0=0):
        """src c-slots (c0, c0+1) --rope--> dst fp8 same slots. [128, T]."""
        i0 = src_bf[:, c0, tok0 : tok0 + T]
        i1 = src_bf[:, c0 + 1, tok0 : tok0 + T]
        co = cosb[:, tok0 : tok0 + T]
        si = sinb[:, tok0 : tok0 + T]
        ta = rsc.tile([128, S], BF16, tag="ta", name="ta")[:, 0:T]
        nc.gpsimd.tensor_mul(ta, i0, co)
        tb = rsc.tile([128, S], BF16, tag="tb", name="tb")[:, 0:T]
        nc.gpsimd.tensor_mul(tb, i1, si)
        nc.gpsimd.tensor_sub(dst8[:, c0, tok0 : tok0 + T], ta, tb)
        tc_ = rsc.tile([128, S], BF16, tag="tc", name="tc")[:, 0:T]
        nc.gpsimd.tensor_mul(tc_, i1, co)
        td = rsc.tile([128, S], BF16, tag="td", name="td")[:, 0:T]
        nc.gpsimd.tensor_mul(td, i0, si)
        nc.gpsimd.tensor_add(dst8[:, c0 + 1, tok0 : tok0 + T], tc_, td)

    # Q: tokens 0..512
    for tt in range(NTQ):
        for fc in range(2):
            pt = qkv_mm(wq_sb, tt, fc, NTQ)
            qsb = s2sb.tile([128, 512], BF16, tag="qsb", name="qsb")
            nc.vector.scalar_tensor_tensor(
                out=qsb, in0=pt, scalar=scl_sb[:, 0:1], in1=bqb[:, fc * 512 : (fc + 1) * 512],
                op0=ALU.mult, op1=ALU.add,
            )
            nc.sync.dma_start_transpose(
                qTb[:, fc * 4 : (fc + 1) * 4, tt * 128 : (tt + 1) * 128], qsb
            )
    wv_sb = wqkvp.tile([128, 4, 2, D], FP8, tag="wv")
    nc.scalar.dma_start(out=wv_sb, in_=wv[:, :, :, :])
    nc.scalar.dma_start(out=bvb, in_=bv[:, :])
    nc.scalar.dma_start(out=b2rb, in_=b2r[:, :])
    nc.scalar.dma_start(out=b1c_sb, in_=b1r[:, :])

    # K: tokens 0..1024
    for tt in range(NT):
        for fc in range(2):
            pt = qkv_mm(wk_sb, tt, fc, NT)
            ksb = s2sb.tile([128, 512], BF16, tag="ksb", name="ksb")
            nc.vector.scalar_tensor_tensor(
                out=ksb, in0=pt, scalar=scl_sb[:, 0:1],
                in1=bkb[:, fc * 512 : (fc + 1) * 512],
                op0=ALU.mult, op1=ALU.add,
            )
            nc.sync.dma_start_transpose(
                kTb[:, fc * 4 : (fc + 1) * 4, tt * 128 : (tt + 1) * 128], ksb
            )
    for u in range(4):
        rope_unit(qTb, qT8, 2 * u, SQ)
    for u in range(4):
        rope_unit(kTb, kT8, 2 * u, S)

    # V: tokens 0..1024 (original head order, +bias, no rope)
    for tt in range(NT):
        for fc in range(2):
            pt = qkv_mm(wv_sb, tt, fc, NT)
            nc.vector.scalar_tensor_tensor(
                out=v8[:, tt // 2, tt % 2, fc * 8 : (fc + 1) * 8, 0:HD],
                in0=pt, scalar=scl_sb[:, 0:1], in1=bvb[:, fc * 512 : (fc + 1) * 512],
                op0=ALU.mult, op1=ALU.add,
            )
    s2ps_cm.__exit__(None, None, None)

    # out-proj + fc2-oc0 weights stream in during attention
    nc.scalar.dma_start(out=wo_sb, in_=wo[:, :, :, :])
    for g in range(8):
        nc.scalar.dma_start(out=w2a_sb[:, 4 * g : 4 * g + 4, :, :],
                            in_=w2[:, 4 * g : 4 * g + 4, :, 0:512])

    # ------------------------------------------------------ stage C: attention
    with (
        tc.tile_pool(name="stpA", bufs=1, space="PSUM") as stpA,
        tc.tile_pool(name="stpB", bufs=1, space="PSUM") as stpB,
        tc.tile_pool(name="pvp", bufs=1, space="PSUM") as pvp,
        tc.tile_pool(name="bcp", bufs=1, space="PSUM") as bcp,
        tc.tile_pool(name="prp", bufs=3) as prp,
        tc.tile_pool(name="s3re", bufs=2) as s3re,
    ):
        for h in range(H):
            fc, hi, hlo = h // 8, (h % 8) // 4, h % 4
            p0 = 32 * hlo
            cq = fc * 4 + 2 * hi
            pv = pvp.tile([HD + 1, 512], FP32, tag="pv", name="pv")
            groups = [(0, 4, stpA, 4), (4, 2, stpB, 2), (6, 2, stpB, 2)]
            for g0, glen, pool, tsz in groups:
                stp = pool.tile([128, tsz, 512], FP32, tag=f"st{tsz}", name="st")
                for u in range(glen):
                    tt = g0 + u
                    nc.tensor.matmul(
                        stp[:, u, :],
                        kT8[p0 : p0 + 32, cq : cq + 2, tt * 128 : (tt + 1) * 128],
                        qT8[p0 : p0 + 32, cq : cq + 2, :],
                        start=True, stop=True,
                        perf_mode=PM.DoubleRow,
                        tile_position=(p0, 0),
                    )
                pr = prp.tile([128, glen, 512], FP8, tag=f"pr{glen}", name="pr")
                nc.scalar.activation(
                    out=pr, in_=stp[:, 0:glen, :], func=AF.Exp, scale=0.125
                )
                for u in range(0, glen, 2):
                    tt = g0 + u
                    nc.tensor.matmul(
                        pv,
                        v8[:, tt // 2, :, h, :],
                        pr[:, u : u + 2, :],
                        start=(tt == 0), stop=(tt == NT - 2),
                        perf_mode=PM.DoubleRow,
                    )
            rec = s3re.tile([1, 512], FP32R, tag="rec")
            with nc.allow_low_precision(reason="softmax denom reciprocal"):
                nc.vector.reciprocal(rec, pv[HD : HD + 1, :])
            bc = bcp.tile([64, 512], FP32, tag="bc", name="bc")
            nc.tensor.matmul(bc, ones64, rec, start=True, stop=True)
            bcs = s3re.tile([64, 512], FP32, tag="bcs")
            nc.vector.tensor_copy(bcs, bc)
            nc.vector.scalar_tensor_tensor(
                out=attnT8[64 * (h % 2) : 64 * (h % 2) + 64, h // 2, :],
                in0=pv[0:HD, :], scalar=1.0, in1=bcs,
                op0=ALU.mult, op1=ALU.mult,
            )

    rsc_cm.__exit__(None, None, None)
    s2sb_cm.__exit__(None, None, None)
    wqkv_cm.__exit__(None, None, None)
    xmT_cm.__exit__(None, None, None)
    qk_cm.__exit__(None, None, None)

    # ------------------------------------------------------ stage D: out-proj
    with tc.tile_pool(name="s4ps", bufs=3, space="PSUM") as s4ps:
        for i in range(NTQ):
            for oc in range(2):
                pt = s4ps.tile([128, 512], FP32, tag="op", name="op")
                for j in range(4):
                    nc.tensor.matmul(
                        pt,
                        attnT8[:, 2 * j : 2 * j + 2, i * 128 : (i + 1) * 128],
                        wo_sb[:, j, :, oc * 512 : (oc + 1) * 512],
                        start=(j == 0), stop=(j == 3),
                        perf_mode=PM.DoubleRow,
                    )
                nc.vector.scalar_tensor_tensor(
                    out=x1[:, i, oc * 512 : (oc + 1) * 512],
                    in0=pt, scalar=scl_sb[:, 1:2],
                    in1=xt[:, i, oc * 512 : (oc + 1) * 512],
                    op0=ALU.mult, op1=ALU.add,
                )
    attnT_cm.__exit__(None, None, None)
    wo_cm.__exit__(None, None, None)

    # ------------------------------------------------------ stage E: LN2
    xm2T_cm = tc.tile_pool(name="xm2Tp", bufs=1)
    xm2Tp = xm2T_cm.__enter__()
    xm2T8 = xm2Tp.tile([128, 8, SQ], FP8, tag="xm2T8")

    with tc.tile_pool(name="s5st", bufs=3) as s5st, \
         tc.tile_pool(name="s5xm", bufs=2) as s5xm, \
         tc.tile_pool(name="s5tb", bufs=2) as s5tb:
        for i in range(NTQ):
            xm2 = s5xm.tile([128, D], BF16, tag="xm2")
            ln_tile(x1[:, i, :], xm2, s5st, veng=nc.vector)
            xm2Tb = s5tb.tile([128, 8, 128], BF16, tag="xm2Tb")
            nc.sync.dma_start_transpose(xm2Tb, xm2)
            nc.vector.tensor_copy(
                xm2T8[:, :, i * 128 : (i + 1) * 128], xm2Tb
            )
        # fold b2*gate_mlp into the residual (after LN2 has read x1)
        for i in range(NTQ):
            nc.gpsimd.tensor_add(x1[:, i, :], x1[:, i, :], b2rb)

    # ------------------------------------------------------ stage F: MLP
    hT_cm = tc.tile_pool(name="hTp", bufs=1)
    hTp = hT_cm.__enter__()
    hT8 = hTp.tile([128, 32, SQ], FP8, tag="hT8")

    pts0_cm = tc.tile_pool(name="pts0", bufs=1, space="PSUM")
    pts0p = pts0_cm.__enter__()
    fc1p_cm = tc.tile_pool(name="fc1p", bufs=3, space="PSUM")
    fc1pp = fc1p_cm.__enter__()

    with tc.tile_pool(name="s6w1", bufs=2) as s6w1, \
         tc.tile_pool(name="s6w2", bufs=16) as s6w2, \
         tc.tile_pool(name="s6o", bufs=3) as s6o:
        pts0 = [pts0p.tile([128, 512], FP32, tag=f"p0_{i}", name=f"p0_{i}")
                for i in range(NTQ)]

        def fc2_step(pts, f, oc):
            if oc == 0:
                w2b = w2a_sb[:, f, :, :]
            else:
                w2b = s6w2.tile([128, 2, 512], FP8, tag="w2b", name="w2b")
                nc.sync.dma_start(out=w2b, in_=w2[:, f, :, 512:1024])
            for i in range(NTQ):
                nc.tensor.matmul(
                    pts[i],
                    hT8[:, f : f + 1, i * 128 : (i + 1) * 128].broadcast_to(
                        [128, 2, 128]
                    ),
                    w2b,
                    start=(f == 0), stop=(f == 31),
                    perf_mode=PM.DoubleRow,
                )

        def fc2_epilogue(pts, oc):
            for i in range(NTQ):
                ot = s6o.tile([128, 512], FP32, tag="outs", name="outs")
                nc.vector.scalar_tensor_tensor(
                    out=ot, in0=pts[i], scalar=scl_sb[:, 3:4],
                    in1=x1[:, i, oc * 512 : (oc + 1) * 512],
                    op0=ALU.mult, op1=ALU.add,
                )
                nc.gpsimd.dma_start(
                    out=outd[i * 128 : (i + 1) * 128, oc * 512 : (oc + 1) * 512],
                    in_=ot,
                )

        for mg in range(8):
            w1s = s6w1.tile([128, 8, 2, 512], FP8, tag="w1s", name="w1s")
            nc.sync.dma_start(out=w1s, in_=w1[:, :, :, mg * 512 : (mg + 1) * 512])
            for mi in range(4):
                mc = mg * 4 + mi
                fp = fc1pp.tile([128, 512], FP32, tag="fp", name="fp")
                for th in range(2):
                    for j in range(8):
                        nc.tensor.matmul(
                            fp[:, th * 256 : (th + 1) * 256],
                            w1s[:, j, :, mi * 128 : (mi + 1) * 128],
                            xm2T8[:, j : j + 1, th * 256 : (th + 1) * 256]
                            .broadcast_to([128, 2, 256]),
                            start=(j == 0), stop=(j == 7),
                            perf_mode=PM.DoubleRow,
                        )
                nc.scalar.activation(
                    out=hT8[:, mc, :], in_=fp,
                    func=AF.Gelu_apprx_tanh, scale=scl_sb[:, 2:3],
                    bias=b1c_sb[:, mc : mc + 1],
                )
            for f in range(4 * mg, 4 * mg + 4):
                fc2_step(pts0, f, 0)
        fc2_epilogue(pts0, 0)
        fc1p_cm.__exit__(None, None, None)

        pts1_cm = tc.tile_pool(name="pts1", bufs=1, space="PSUM")
        pts1p = pts1_cm.__enter__()
        pts1 = [pts1p.tile([128, 512], FP32, tag=f"p1_{i}", name=f"p1_{i}")
                for i in range(NTQ)]
        for f in range(32):
            fc2_step(pts1, f, 1)
        fc2_epilogue(pts1, 1)
        pts1_cm.__exit__(None, None, None)

    pts0_cm.__exit__(None, None, None)
    hT_cm.__exit__(None, None, None)
    xm2T_cm.__exit__(None, None, None)
    w2a_cm.__exit__(None, None, None)
    x1_cm.__exit__(None, None, None)
    x_cm.__exit__(None, None, None)
    rows_cm.__exit__(None, None, None)
    const_cm.__exit__(None, None, None)


_NC_CACHE = {}


def _get_nc(reps=1):
    if reps not in _NC_CACHE:
        _NC_CACHE[reps] = _build_nc(reps)
    return _NC_CACHE[reps]


# ---------------------------------------------------------------------------
# Host-side folding / packing

def _pack_2lvl(w, s):
    """[1024|4096, N] -> [128, nk, 2, N] fp8 pairs (hi, lo) per 128-row chunk."""
    ws = w * s
    hi = ws.astype(NP_FP8)
    lo = (ws - hi.astype(np.float32)).astype(NP_FP8)
    nk = w.shape[0] // 128
    out = np.empty((128, nk, 2, w.shape[1]), dtype=NP_FP8)
    hi = hi.reshape(nk, 128, -1)
    lo = lo.reshape(nk, 128, -1)
    out[:, :, 0, :] = hi.transpose(1, 0, 2)
    out[:, :, 1, :] = lo.transpose(1, 0, 2)
    return np.ascontiguousarray(out)


def _pow2_scale(w, target=192.0):
    """Largest power of two s with absmax(w)*s <= target."""
    a = float(np.max(np.abs(w)))
    if a == 0:
        return 1.0
    return 2.0 ** int(np.floor(np.log2(target / a)))

# head-permuted column order for Q/K: j(h', half, i) within an fc-chunk
#   j = (h'//4)*256 + half*128 + (h'%4)*32 + i      (h' = head within fc)


def _qk_perm():
    """perm[j_new] = j_orig for one 512-wide fc chunk."""
    p = np.zeros(512, dtype=np.int64)
    for hp in range(8):
        hi, hlo = hp // 4, hp % 4
        for half in range(2):
            for i in range(32):
                jn = hi * 256 + half * 128 + hlo * 32 + i
                p[jn] = hp * 64 + half * 32 + i
    return p


_QKPERM = _qk_perm()
_QKPERM2 = np.concatenate([_QKPERM, 512 + _QKPERM])  # both fc chunks


def _pack_kpairs(w):
    """[1024, N] -> [128, 4, 2, N] with rows d = (2j+t)*128 + p."""
    n = w.shape[1]
    return np.ascontiguousarray(
        w.reshape(4, 2, 128, n).transpose(2, 0, 1, 3)
    )


def _f8(a):
    return np.ascontiguousarray(np.asarray(a, dtype=np.float32)).astype(NP_FP8)


def _bf(a):
    return np.ascontiguousarray(np.asarray(a, dtype=np.float32)).astype(NP_BF16)


def _make_in_maps(x, c, norm1_w, norm2_w, w_qkv, w_out, w1, b1, w2, b2,
                  adaLN_w, adaLN_b, cos, sin):
    f32 = lambda a: np.asarray(a, dtype=np.float32)
    x = f32(x); c = f32(c)
    w_qkv = f32(w_qkv); w_out = f32(w_out); w1 = f32(w1); w2 = f32(w2)
    cos = f32(cos); sin = f32(sin)

    # adaLN modulation on host: [B, 6D]
    mod = c @ f32(adaLN_w) + f32(adaLN_b)
    sh_msa, sc_msa, g_msa, sh_mlp, sc_mlp, g_mlp = np.split(mod, 6, axis=-1)

    in_maps = []
    for core in range(N_CORES):
        b, half = core // 2, core % 2
        sh = -half * SQ

        m1 = (1.0 + sc_msa[b]) * f32(norm1_w)      # [D]
        m2 = (1.0 + sc_mlp[b]) * f32(norm2_w)
        wqkv_s = w_qkv * m1[:, None]               # [D, 3D]
        biasqkv = sh_msa[b] @ w_qkv                # [3D]

        wq_c = wqkv_s[:, 0:D][:, _QKPERM2]
        wk_c = wqkv_s[:, D : 2 * D][:, _QKPERM2]
        wv_c = wqkv_s[:, 2 * D : 3 * D]
        bq_c = biasqkv[0:D][_QKPERM2]
        bk_c = biasqkv[D : 2 * D][_QKPERM2]
        bv_c = biasqkv[2 * D : 3 * D]

        # wout: columns gated; rows follow the attnT8 partition layout
        wout_s = w_out * g_msa[b][None, :]
        dmap = np.zeros(D, dtype=np.int64)
        for p in range(128):
            for s in range(8):
                dmap[s * 128 + p] = (2 * s + p // 64) * 64 + (p % 64)
        wout_p = wout_s[dmap, :]

        w1_s = w1 * m2[:, None]
        bias1 = sh_mlp[b] @ w1 + f32(b1)
        w2_s = w2 * g_mlp[b][None, :]
        b2g = f32(b2) * g_mlp[b]

        sqkv = _pow2_scale(wqkv_s)
        swo = _pow2_scale(wout_p)
        sw1 = _pow2_scale(w1_s)
        sw2 = _pow2_scale(w2_s)
        sclv = np.zeros(8, dtype=np.float32)
        sclv[0], sclv[1], sclv[2], sclv[3] = 1/sqkv, 1/swo, 1/sw1, 1/sw2

        # rope tables, rotated with the tokens: row p -> freq index p%32
        idx = (np.arange(S) + half * SQ) % S
        cosr = cos[idx][:, None, :]                # [S, 1, 32]
        cosT = np.tile(cosr.transpose(2, 1, 0).reshape(32, S), (4, 1))
        sinr = sin[idx][:, None, :]
        sinT = np.tile(sinr.transpose(2, 1, 0).reshape(32, S), (4, 1))

        in_maps.append(dict(
            xb=_bf(np.roll(x[b], sh, axis=0)),
            wq=_f8(_pack_kpairs(wq_c * sqkv)),
            wk=_f8(_pack_kpairs(wk_c * sqkv)),
            wv=_f8(_pack_kpairs(wv_c * sqkv)),
            wo=_f8(_pack_kpairs(wout_p * swo)),
            w1=_pack_2lvl(w1_s, sw1),
            w2=_pack_2lvl(w2_s, sw2),
            scl=np.ascontiguousarray(np.tile(sclv[None, :], (128, 1))),
            cosT=_bf(cosT), sinT=_bf(sinT),
            bq=_bf(np.tile(bq_c[None, :], (128, 1))),
            bk=_bf(np.tile(bk_c[None, :], (128, 1))),
            bv=_bf(np.tile(bv_c[None, :], (128, 1))),
            b2r=_bf(np.tile(b2g[None, :], (128, 1))),
            b1r=np.ascontiguousarray(bias1.reshape(32, 128).T.astype(np.float32)),
        ))
    return in_maps


def _gather(results, x_shape):
    B = x_shape[0]
    out = np.empty(x_shape, dtype=np.float32)
    for core in range(N_CORES):
        b, half = core // 2, core % 2
        out[b, half * SQ : (half + 1) * SQ] = results[core]["out"]
    return out


def run(inputs, trace=False, reps=1):
    nc = _get_nc(reps)
    in_maps = _make_in_maps(**inputs)
    res = run_bass_kernel_spmd(nc, in_maps, list(range(N_CORES)), trace=trace)
    out = _gather(res.results, np.asarray(inputs["x"]).shape)
    return out, res


def kernel(**inputs):
    out, _ = run(inputs)
    return out


# revision 50
# speedup vs baseline: 1.0048x; 1.0048x over previous
"""DiT block kernel for 8 Trainium2 NeuronCores — fp8 DoubleRow redesign.

Sharding: data-parallel over (batch, seq-half) -> 8 shards, no collectives.
Each core gets x[b] rotated so its 512 query tokens are rows 0:511; K/V are
computed over the full (rotated) 1024-token sequence.

Key ideas vs the fp32r baseline:
- The adaLN modulation (a [1,128]@[128,6144] matmul) is computed on the HOST
  inside kernel(); (1+scale)*norm_w folds into weight rows, shifts into bias
  rows, gates into w_out/w2 columns. The device kernel runs plain LayerNorm.
- All large matmuls run as fp8e4m3 DoubleRow (2 k-tiles of 128 per
  instruction at 0.5 cycles/row): qkv, scores, PV, out-proj, fc1, fc2.
  Weights are host-quantized with power-of-2 scales (descaled for free in
  the fused psum-export ops / gelu scale).
- Transposes go through the XBAR DMA engine (dma_start_transpose, bf16) on
  the SP queue instead of the PE array; rope runs after the transpose in a
  head-permuted column layout (partner halves land in adjacent c-slots, so
  rope is full-128-partition work on Pool and writes fp8 directly).
- LayerNorm stats via bn_stats/bn_aggr on DVE; normalize+cast on Pool.
"""

import sys

sys.path.insert(0, "/opt/trn_rl_repo")

import numpy as np
import ml_dtypes

import concourse.bass as bass
import concourse.mybir as mybir
from concourse.bass_utils import run_bass_kernel_spmd
from concourse.tile import TileContext
from concourse.vector_clock import ScopedClock, VectorClock

# ---------------------------------------------------------------------------
# Walrus in this container caps sync-wait commands per CTRL instruction at a
# small number; Tile's stock tail drain collects one wait per live proc and
# trips that cap. Split the final waits across one SP NOP per proc instead.
_orig_drain_and_barrier = TileContext._drain_and_barrier


def _split_drain_and_barrier(self, tick_clock, wait_clock):
    gc_list = list(tick_clock.global_clock)
    for p, tick in enumerate(gc_list):
        if tick > 0:
            partial = [0] * len(gc_list)
            partial[p] = tick
            nop = self.nc.sync.nop()
            wait_clock.add_sem_waits(nop.ins, ScopedClock({None: VectorClock(partial)}))
    drain_inst = self.nc.sync.drain()
    req = ScopedClock({None: tick_clock.global_clock.copy()})
    cur = ScopedClock({None: tick_clock.global_clock.copy()})
    wait_clock.add_sem_waits(drain_inst.ins, req, cur)
    self.nc.all_engine_barrier()
    popped = self.nc._tile_sem_poison_stack.pop()
    assert popped is self._sem_poison
    self.nc.clear_and_free_semaphores(list(self.sems.allocated().values()))
    self.nc.all_engine_barrier()


TileContext._drain_and_barrier = _split_drain_and_barrier

# This walrus also caps waits per *compute/DMA* instruction (the S3_LW struct
# allows a single wait). Intercept every instruction Tile commits to a basic
# block and spill all but the last wait onto preceding same-engine NOPs.
_nop_proto = None


def _get_nop_proto():
    global _nop_proto
    if _nop_proto is None:
        scratch = bass.Bass()
        _nop_proto = scratch.sync.nop().ins
        _nop_proto.sync_info = None
    return _nop_proto


_orig_add_instruction = TileContext._add_instruction


def _add_instruction_capped(self, inst):
    si = inst.sync_info
    if si is not None and si.on_wait is not None and len(si.on_wait) > 1:
        waits = list(si.on_wait)
        si.on_wait = waits[-1:]
        import copy as _copy

        for w in waits[:-1]:
            nop = _copy.deepcopy(_get_nop_proto())
            nop.name = self.nc.get_next_instruction_name()
            nop.engine = inst.engine
            nop.sync_info = mybir.SyncInfo(on_wait=[w], on_update=[])
            _orig_add_instruction(self, nop)
    _orig_add_instruction(self, inst)


TileContext._add_instruction = _add_instruction_capped

# Capture the Tile scheduler's modeled makespan (cost-model ns) per block —
# the only timing signal available in this container (no NTFF profiling).
MODELED_NS = []
from concourse.bass_interp import CoreSim as _CoreSim

_orig_coresim_simulate = _CoreSim.simulate


def _simulate_capture(self, *a, **k):
    r = _orig_coresim_simulate(self, *a, **k)
    try:
        if self.is_scheduling_pass():
            MODELED_NS.append((getattr(self, "name", "?"), int(self.time)))
    except Exception:
        pass
    return r


_CoreSim.simulate = _simulate_capture


def _publish_perfetto_local(self):
    """Dump the scheduling-sim perfetto trace to a local file (no upload)."""
    import os as _os

    if self.perfetto is None:
        return
    path = _os.environ.get("BASS_MODEL_TRACE_PATH", "/tmp/model_trace.pftrace")
    with open(path, "wb") as f:
        f.write(bytes(self.perfetto.take_serialized()))
    print(f"[kernel] modeled trace written to {path}")


_CoreSim.publish_perfetto = _publish_perfetto_local

# ---------------------------------------------------------------------------

FP32 = mybir.dt.float32
FP32R = mybir.dt.float32r
FP8 = mybir.dt.float8e4
BF16 = mybir.dt.bfloat16
AF = mybir.ActivationFunctionType
ALU = mybir.AluOpType
PM = mybir.MatmulPerfMode

D = 1024
H = 16
HD = 64
S = 1024
SQ = 512  # query tokens per core
MLP = 4096
NT = S // 128  # 8 token tiles
NTQ = SQ // 128  # 4 query token tiles
LN_EPS = 1e-5
N_CORES = 8

NP_FP8 = ml_dtypes.float8_e4m3
NP_BF16 = ml_dtypes.bfloat16


def _build_nc(reps=1):
    import os as _os

    trace_sim = bool(_os.environ.get("BASS_MODEL_TRACE"))
    nc = bass.Bass()

    xb = nc.dram_tensor("xb", [S, D], BF16, kind="ExternalInput")
    wq = nc.dram_tensor("wq", [128, 4, 2, D], FP8, kind="ExternalInput")
    wk = nc.dram_tensor("wk", [128, 4, 2, D], FP8, kind="ExternalInput")
    wv = nc.dram_tensor("wv", [128, 4, 2, D], FP8, kind="ExternalInput")
    wo = nc.dram_tensor("wo", [128, 4, 2, D], FP8, kind="ExternalInput")
    w1 = nc.dram_tensor("w1", [128, 8, 2, MLP], FP8, kind="ExternalInput")
    w2 = nc.dram_tensor("w2", [128, 32, 2, D], FP8, kind="ExternalInput")
    cosT = nc.dram_tensor("cosT", [128, S], BF16, kind="ExternalInput")
    sinT = nc.dram_tensor("sinT", [128, S], BF16, kind="ExternalInput")
    bq = nc.dram_tensor("bq", [128, D], BF16, kind="ExternalInput")
    bk = nc.dram_tensor("bk", [128, D], BF16, kind="ExternalInput")
    bv = nc.dram_tensor("bv", [128, D], BF16, kind="ExternalInput")
    b2r = nc.dram_tensor("b2r", [128, D], BF16, kind="ExternalInput")
    b1r = nc.dram_tensor("b1r", [128, 32], FP32, kind="ExternalInput")
    scl = nc.dram_tensor("scl", [128, 8], FP32, kind="ExternalInput")
    outd = nc.dram_tensor("out", [SQ, D], FP32, kind="ExternalOutput")

    with TileContext(nc, trace_sim=trace_sim) as tc:
        for _rep in range(reps):
            _emit_body(nc, tc, xb, wq, wk, wv, wo, w1, w2, cosT, sinT,
                       bq, bk, bv, b2r, b1r, scl, outd)
    return nc


def _emit_body(nc, tc, xb, wq, wk, wv, wo, w1, w2, cosT, sinT,
               bq, bk, bv, b2r, b1r, scl, outd):
    # ------------------------------------------------------------ persistent
    const_cm = tc.tile_pool(name="const", bufs=1)
    const = const_cm.__enter__()
    epst = const.tile([128, 1], FP32, tag="eps")
    nc.vector.memset(epst, LN_EPS)
    ones64f = const.tile([1, 64], FP32, tag="o64f")
    nc.vector.memset(ones64f, 1.0)
    ones64 = const.tile([1, 64], FP32R, tag="o64")
    nc.vector.tensor_copy(ones64, ones64f)

    rows_cm = tc.tile_pool(name="rows", bufs=1)
    rows = rows_cm.__enter__()
    scl_sb = rows.tile([128, 8], FP32, tag="scl")
    nc.scalar.dma_start(out=scl_sb, in_=scl[:, :])
    bqb = rows.tile([128, D], BF16, tag="bqb")
    nc.scalar.dma_start(out=bqb, in_=bq[:, :])
    bkb = rows.tile([128, D], BF16, tag="bkb")
    nc.scalar.dma_start(out=bkb, in_=bk[:, :])
    cosb = rows.tile([128, S], BF16, tag="cosb")
    sinb = rows.tile([128, S], BF16, tag="sinb")
    bvb = rows.tile([128, D], BF16, tag="bvb")
    b2rb = rows.tile([128, D], BF16, tag="b2rb")
    b1c_sb = rows.tile([128, 32], FP32, tag="b1c")

    # x tiles (kept resident: tiles 0-3 are reused as the residual in s4)
    x_cm = tc.tile_pool(name="xp", bufs=1)
    xp = x_cm.__enter__()
    xt = xp.tile([128, NT, D], BF16, tag="xt")
    for tt in range(NT):
        nc.sync.dma_start(out=xt[:, tt, :], in_=xb[tt * 128 : (tt + 1) * 128, :])

    # long-lived pools opened early so pool lifetimes nest (LIFO releases)
    x1_cm = tc.tile_pool(name="x1p", bufs=1)
    x1p = x1_cm.__enter__()
    x1 = x1p.tile([128, NTQ, D], FP32, tag="x1")
    w2a_cm = tc.tile_pool(name="w2ap", bufs=1)
    w2ap = w2a_cm.__enter__()
    w2a_sb = w2ap.tile([128, 32, 2, 512], FP8, tag="w2a")
    wo_cm = tc.tile_pool(name="wop", bufs=1)
    wop = wo_cm.__enter__()
    wo_sb = wop.tile([128, 4, 2, D], FP8, tag="wo")
    attnT_cm = tc.tile_pool(name="attnTp", bufs=1)
    attnTp = attnT_cm.__enter__()
    attnT8 = attnTp.tile([128, 8, SQ], FP8, tag="attnT8")

    qk_cm = tc.tile_pool(name="qkp", bufs=1)
    qkp = qk_cm.__enter__()
    qTb = qkp.tile([128, 8, SQ], BF16, tag="qTb")
    kTb = qkp.tile([128, 8, S], BF16, tag="kTb")
    qT8 = qkp.tile([128, 8, SQ], FP8, tag="qT8")
    kT8 = qkp.tile([128, 8, S], FP8, tag="kT8")
    v8 = qkp.tile([128, 4, 2, H, HD + 1], FP8, tag="v8")
    nc.gpsimd.memset(v8[:, :, :, :, HD : HD + 1], 1.0)

    # transposed-activation pool (released after V)
    xmT_cm = tc.tile_pool(name="xmTp", bufs=1)
    xmTp = xmT_cm.__enter__()
    xmT8 = xmTp.tile([128, 8, S], FP8, tag="xmT8")

    # q/k/v weights (one pool, released after V)
    wqkv_cm = tc.tile_pool(name="wqkvp", bufs=1)
    wqkvp = wqkv_cm.__enter__()
    wq_sb = wqkvp.tile([128, 4, 2, D], FP8, tag="wq")
    nc.scalar.dma_start(out=wq_sb, in_=wq[:, :, :, :])
    wk_sb = wqkvp.tile([128, 4, 2, D], FP8, tag="wk")
    nc.scalar.dma_start(out=wk_sb, in_=wk[:, :, :, :])
    nc.scalar.dma_start(out=cosb, in_=cosT[:, :])
    nc.scalar.dma_start(out=sinb, in_=sinT[:, :])

    # ------------------------------------------------------------ LN helper
    def ln_tile(src_ap, dst_bf_ap, st, veng=nc.gpsimd):
        """LayerNorm(src) -> dst (bf16). src/dst [128, 1024] APs."""
        st6 = st.tile([128, 2, 6], FP32, tag="st6")
        srcv = src_ap.rearrange("p (a b) -> p a b", a=2)
        nc.vector.bn_stats(st6[:, 0, :], srcv[:, 0, :])
        nc.vector.bn_stats(st6[:, 1, :], srcv[:, 1, :])
        agg = st.tile([128, 2], FP32, tag="agg")
        nc.vector.bn_aggr(agg, st6)
        std = st.tile([128, 1], FP32, tag="std")
        nc.scalar.activation(out=std, in_=agg[:, 1:2], func=AF.Sqrt, bias=epst)
        rstd = st.tile([128, 1], FP32, tag="rstd")
        nc.vector.reciprocal(rstd, std)
        veng.tensor_scalar(
            out=dst_bf_ap, in0=src_ap, scalar1=agg[:, 0:1], scalar2=rstd,
            op0=ALU.subtract, op1=ALU.mult,
        )

    # ------------------------------------------------------ stage A: LN1
    with tc.tile_pool(name="s1st", bufs=3) as s1st, \
         tc.tile_pool(name="s1xm", bufs=2) as s1xm, \
         tc.tile_pool(name="s1tb", bufs=3) as s1tb:
        for tt in range(NT):
            xm = s1xm.tile([128, D], BF16, tag="xm")
            ln_tile(xt[:, tt, :], xm, s1st)
            xmTb = s1tb.tile([128, 8, 128], BF16, tag="xmTb")
            nc.sync.dma_start_transpose(xmTb, xm)
            nc.vector.tensor_copy(
                xmT8[:, :, tt * 128 : (tt + 1) * 128], xmTb
            )

    # ------------------------------------------------------ stage B: QKV
    s2ps_cm = tc.tile_pool(name="s2ps", bufs=4, space="PSUM")
    s2ps = s2ps_cm.__enter__()
    s2sb_cm = tc.tile_pool(name="s2sb", bufs=3)
    s2sb = s2sb_cm.__enter__()
    rsc_cm = tc.tile_pool(name="rsc", bufs=2)
    rsc = rsc_cm.__enter__()

    def qkv_mm(w_sb, tt, fc, nt_tok):
        """One [128tok, 512] qkv psum tile via 4 DoubleRow matmuls."""
        pt = s2ps.tile([128, 512], FP32, tag="qkvp", name="qkvp")
        for j in range(4):
            nc.tensor.matmul(
                pt,
                xmT8[:, 2 * j : 2 * j + 2, tt * 128 : (tt + 1) * 128],
                w_sb[:, j, :, fc * 512 : (fc + 1) * 512],
                start=(j == 0), stop=(j == 3),
                perf_mode=PM.DoubleRow,
            )
        return pt

    def rope_unit(src_bf, dst8, c0, T, tok0=0):
        """src c-slots (c0, c0+1) --rope--> dst fp8 same slots. [128, T]."""
        i0 = src_bf[:, c0, tok0 : tok0 + T]
        i1 = src_bf[:, c0 + 1, tok0 : tok0 + T]
        co = cosb[:, tok0 : tok0 + T]
        si = sinb[:, tok0 : tok0 + T]
        ta = rsc.tile([128, S], BF16, tag="ta", name="ta")[:, 0:T]
        nc.gpsimd.tensor_mul(ta, i0, co)
        tb = rsc.tile([128, S], BF16, tag="tb", name="tb")[:, 0:T]
        nc.gpsimd.tensor_mul(tb, i1, si)
        nc.gpsimd.tensor_sub(dst8[:, c0, tok0 : tok0 + T], ta, tb)
        tc_ = rsc.tile([128, S], BF16, tag="tc", name="tc")[:, 0:T]
        nc.gpsimd.tensor_mul(tc_, i1, co)
        td = rsc.tile([128, S], BF16, tag="td", name="td")[:, 0:T]
        nc.gpsimd.tensor_mul(td, i0, si)
        nc.gpsimd.tensor_add(dst8[:, c0 + 1, tok0 : tok0 + T], tc_, td)

    # Q: tokens 0..512
    for tt in range(NTQ):
        for fc in range(2):
            pt = qkv_mm(wq_sb, tt, fc, NTQ)
            qsb = s2sb.tile([128, 512], BF16, tag="qsb", name="qsb")
            nc.vector.scalar_tensor_tensor(
                out=qsb, in0=pt, scalar=scl_sb[:, 0:1], in1=bqb[:, fc * 512 : (fc + 1) * 512],
                op0=ALU.mult, op1=ALU.add,
            )
            nc.sync.dma_start_transpose(
                qTb[:, fc * 4 : (fc + 1) * 4, tt * 128 : (tt + 1) * 128], qsb
            )
    wv_sb = wqkvp.tile([128, 4, 2, D], FP8, tag="wv")
    nc.scalar.dma_start(out=wv_sb, in_=wv[:, :, :, :])
    nc.scalar.dma_start(out=bvb, in_=bv[:, :])
    nc.scalar.dma_start(out=b2rb, in_=b2r[:, :])
    nc.scalar.dma_start(out=b1c_sb, in_=b1r[:, :])

    # K: tokens 0..1024
    for tt in range(NT):
        for fc in range(2):
            pt = qkv_mm(wk_sb, tt, fc, NT)
            ksb = s2sb.tile([128, 512], BF16, tag="ksb", name="ksb")
            nc.vector.scalar_tensor_tensor(
                out=ksb, in0=pt, scalar=scl_sb[:, 0:1],
                in1=bkb[:, fc * 512 : (fc + 1) * 512],
                op0=ALU.mult, op1=ALU.add,
            )
            nc.sync.dma_start_transpose(
                kTb[:, fc * 4 : (fc + 1) * 4, tt * 128 : (tt + 1) * 128], ksb
            )
    for u in range(4):
        rope_unit(qTb, qT8, 2 * u, SQ)
    for u in range(4):
        rope_unit(kTb, kT8, 2 * u, S)

    # V: tokens 0..1024 (original head order, +bias, no rope)
    for tt in range(NT):
        for fc in range(2):
            pt = qkv_mm(wv_sb, tt, fc, NT)
            nc.vector.scalar_tensor_tensor(
                out=v8[:, tt // 2, tt % 2, fc * 8 : (fc + 1) * 8, 0:HD],
                in0=pt, scalar=scl_sb[:, 0:1], in1=bvb[:, fc * 512 : (fc + 1) * 512],
                op0=ALU.mult, op1=ALU.add,
            )
    s2ps_cm.__exit__(None, None, None)

    # out-proj + fc2-oc0 weights stream in during attention
    nc.scalar.dma_start(out=wo_sb, in_=wo[:, :, :, :])
    for g in range(8):
        nc.scalar.dma_start(out=w2a_sb[:, 4 * g : 4 * g + 4, :, :],
                            in_=w2[:, 4 * g : 4 * g + 4, :, 0:512])

    # ------------------------------------------------------ stage C: attention
    with (
        tc.tile_pool(name="stpA", bufs=1, space="PSUM") as stpA,
        tc.tile_pool(name="stpB", bufs=1, space="PSUM") as stpB,
        tc.tile_pool(name="pvp", bufs=1, space="PSUM") as pvp,
        tc.tile_pool(name="bcp", bufs=1, space="PSUM") as bcp,
        tc.tile_pool(name="prp", bufs=3) as prp,
        tc.tile_pool(name="s3re", bufs=2) as s3re,
    ):
        for h in range(H):
            fc, hi, hlo = h // 8, (h % 8) // 4, h % 4
            p0 = 32 * hlo
            cq = fc * 4 + 2 * hi
            pv = pvp.tile([HD + 1, 512], FP32, tag="pv", name="pv")
            groups = [(0, 4, stpA, 4), (4, 2, stpB, 2), (6, 2, stpB, 2)]
            for g0, glen, pool, tsz in groups:
                stp = pool.tile([128, tsz, 512], FP32, tag=f"st{tsz}", name="st")
                for u in range(glen):
                    tt = g0 + u
                    nc.tensor.matmul(
                        stp[:, u, :],
                        kT8[p0 : p0 + 32, cq : cq + 2, tt * 128 : (tt + 1) * 128],
                        qT8[p0 : p0 + 32, cq : cq + 2, :],
                        start=True, stop=True,
                        perf_mode=PM.DoubleRow,
                        tile_position=(p0, 0),
                    )
                pr = prp.tile([128, glen, 512], FP8, tag=f"pr{glen}", name="pr")
                nc.scalar.activation(
                    out=pr, in_=stp[:, 0:glen, :], func=AF.Exp, scale=0.125
                )
                for u in range(0, glen, 2):
                    tt = g0 + u
                    nc.tensor.matmul(
                        pv,
                        v8[:, tt // 2, :, h, :],
                        pr[:, u : u + 2, :],
                        start=(tt == 0), stop=(tt == NT - 2),
                        perf_mode=PM.DoubleRow,
                    )
            rec = s3re.tile([1, 512], FP32R, tag="rec")
            with nc.allow_low_precision(reason="softmax denom reciprocal"):
                nc.vector.reciprocal(rec, pv[HD : HD + 1, :])
            bc = bcp.tile([64, 512], FP32, tag="bc", name="bc")
            nc.tensor.matmul(bc, ones64, rec, start=True, stop=True)
            bcs = s3re.tile([64, 512], FP32, tag="bcs")
            nc.vector.tensor_copy(bcs, bc)
            nc.vector.scalar_tensor_tensor(
                out=attnT8[64 * (h % 2) : 64 * (h % 2) + 64, h // 2, :],
                in0=pv[0:HD, :], scalar=1.0, in1=bcs,
                op0=ALU.mult, op1=ALU.mult,
            )

    rsc_cm.__exit__(None, None, None)
    s2sb_cm.__exit__(None, None, None)
    wqkv_cm.__exit__(None, None, None)
    xmT_cm.__exit__(None, None, None)
    qk_cm.__exit__(None, None, None)

    # ------------------------------------------------------ stage D: out-proj
    with tc.tile_pool(name="s4ps", bufs=4, space="PSUM") as s4ps:
        for i in range(NTQ):
            for oc in range(2):
                pt = s4ps.tile([128, 512], FP32, tag="op", name="op")
                for j in range(4):
                    nc.tensor.matmul(
                        pt,
                        attnT8[:, 2 * j : 2 * j + 2, i * 128 : (i + 1) * 128],
                        wo_sb[:, j, :, oc * 512 : (oc + 1) * 512],
                        start=(j == 0), stop=(j == 3),
                        perf_mode=PM.DoubleRow,
                    )
                nc.vector.scalar_tensor_tensor(
                    out=x1[:, i, oc * 512 : (oc + 1) * 512],
                    in0=pt, scalar=scl_sb[:, 1:2],
                    in1=xt[:, i, oc * 512 : (oc + 1) * 512],
                    op0=ALU.mult, op1=ALU.add,
                )
    attnT_cm.__exit__(None, None, None)
    wo_cm.__exit__(None, None, None)

    # ------------------------------------------------------ stage E: LN2
    xm2T_cm = tc.tile_pool(name="xm2Tp", bufs=1)
    xm2Tp = xm2T_cm.__enter__()
    xm2T8 = xm2Tp.tile([128, 8, SQ], FP8, tag="xm2T8")

    with tc.tile_pool(name="s5st", bufs=3) as s5st, \
         tc.tile_pool(name="s5xm", bufs=3) as s5xm, \
         tc.tile_pool(name="s5tb", bufs=3) as s5tb:
        for i in range(NTQ):
            xm2 = s5xm.tile([128, D], BF16, tag="xm2")
            ln_tile(x1[:, i, :], xm2, s5st, veng=nc.vector)
            xm2Tb = s5tb.tile([128, 8, 128], BF16, tag="xm2Tb")
            nc.sync.dma_start_transpose(xm2Tb, xm2)
            nc.vector.tensor_copy(
                xm2T8[:, :, i * 128 : (i + 1) * 128], xm2Tb
            )
        # fold b2*gate_mlp into the residual (after LN2 has read x1)
        for i in range(NTQ):
            nc.gpsimd.tensor_add(x1[:, i, :], x1[:, i, :], b2rb)

    # ------------------------------------------------------ stage F: MLP
    hT_cm = tc.tile_pool(name="hTp", bufs=1)
    hTp = hT_cm.__enter__()
    hT8 = hTp.tile([128, 32, SQ], FP8, tag="hT8")

    pts0_cm = tc.tile_pool(name="pts0", bufs=1, space="PSUM")
    pts0p = pts0_cm.__enter__()
    fc1p_cm = tc.tile_pool(name="fc1p", bufs=3, space="PSUM")
    fc1pp = fc1p_cm.__enter__()

    with tc.tile_pool(name="s6w1", bufs=2) as s6w1, \
         tc.tile_pool(name="s6w2", bufs=16) as s6w2, \
         tc.tile_pool(name="s6o", bufs=3) as s6o:
        pts0 = [pts0p.tile([128, 512], FP32, tag=f"p0_{i}", name=f"p0_{i}")
                for i in range(NTQ)]

        def fc2_step(pts, f, oc):
            if oc == 0:
                w2b = w2a_sb[:, f, :, :]
            else:
                w2b = s6w2.tile([128, 2, 512], FP8, tag="w2b", name="w2b")
                nc.sync.dma_start(out=w2b, in_=w2[:, f, :, 512:1024])
            for i in range(NTQ):
                nc.tensor.matmul(
                    pts[i],
                    hT8[:, f : f + 1, i * 128 : (i + 1) * 128].broadcast_to(
                        [128, 2, 128]
                    ),
                    w2b,
                    start=(f == 0), stop=(f == 31),
                    perf_mode=PM.DoubleRow,
                )

        def fc2_epilogue(pts, oc):
            for i in range(NTQ):
                ot = s6o.tile([128, 512], FP32, tag="outs", name="outs")
                nc.vector.scalar_tensor_tensor(
                    out=ot, in0=pts[i], scalar=scl_sb[:, 3:4],
                    in1=x1[:, i, oc * 512 : (oc + 1) * 512],
                    op0=ALU.mult, op1=ALU.add,
                )
                nc.gpsimd.dma_start(
                    out=outd[i * 128 : (i + 1) * 128, oc * 512 : (oc + 1) * 512],
                    in_=ot,
                )

        for mg in range(8):
            w1s = s6w1.tile([128, 8, 2, 512], FP8, tag="w1s", name="w1s")
            nc.sync.dma_start(out=w1s, in_=w1[:, :, :, mg * 512 : (mg + 1) * 512])
            for mi in range(4):
                mc = mg * 4 + mi
                fp = fc1pp.tile([128, 512], FP32, tag="fp", name="fp")
                for th in range(2):
                    for j in range(8):
                        nc.tensor.matmul(
                            fp[:, th * 256 : (th + 1) * 256],
                            w1s[:, j, :, mi * 128 : (mi + 1) * 128],
                            xm2T8[:, j : j + 1, th * 256 : (th + 1) * 256]
                            .broadcast_to([128, 2, 256]),
                            start=(j == 0), stop=(j == 7),
                            perf_mode=PM.DoubleRow,
                        )
                nc.scalar.activation(
                    out=hT8[:, mc, :], in_=fp,
                    func=AF.Gelu_apprx_tanh, scale=scl_sb[:, 2:3],
                    bias=b1c_sb[:, mc : mc + 1],
                )
            for f in range(4 * mg, 4 * mg + 4):
                fc2_step(pts0, f, 0)
        fc2_epilogue(pts0, 0)
        fc1p_cm.__exit__(None, None, None)

        pts1_cm = tc.tile_pool(name="pts1", bufs=1, space="PSUM")
        pts1p = pts1_cm.__enter__()
        pts1 = [pts1p.tile([128, 512], FP32, tag=f"p1_{i}", name=f"p1_{i}")
                for i in range(NTQ)]
        for f in range(32):
            fc2_step(pts1, f, 1)
        fc2_epilogue(pts1, 1)
        pts1_cm.__exit__(None, None, None)

    pts0_cm.__exit__(None, None, None)
    hT_cm.__exit__(None, None, None)
    xm2T_cm.__exit__(None, None, None)
    w2a_cm.__exit__(None, None, None)
    x1_cm.__exit__(None, None, None)
    x_cm.__exit__(None, None, None)
    rows_cm.__exit__(None, None, None)
    const_cm.__exit__(None, None, None)


_NC_CACHE = {}


def _get_nc(reps=1):
    if reps not in _NC_CACHE:
        _NC_CACHE[reps] = _build_nc(reps)
    return _NC_CACHE[reps]


# ---------------------------------------------------------------------------
# Host-side folding / packing

def _pack_2lvl(w, s):
    """[1024|4096, N] -> [128, nk, 2, N] fp8 pairs (hi, lo) per 128-row chunk."""
    ws = w * s
    hi = ws.astype(NP_FP8)
    lo = (ws - hi.astype(np.float32)).astype(NP_FP8)
    nk = w.shape[0] // 128
    out = np.empty((128, nk, 2, w.shape[1]), dtype=NP_FP8)
    hi = hi.reshape(nk, 128, -1)
    lo = lo.reshape(nk, 128, -1)
    out[:, :, 0, :] = hi.transpose(1, 0, 2)
    out[:, :, 1, :] = lo.transpose(1, 0, 2)
    return np.ascontiguousarray(out)


def _pow2_scale(w, target=192.0):
    """Largest power of two s with absmax(w)*s <= target."""
    a = float(np.max(np.abs(w)))
    if a == 0:
        return 1.0
    return 2.0 ** int(np.floor(np.log2(target / a)))

# head-permuted column order for Q/K: j(h', half, i) within an fc-chunk
#   j = (h'//4)*256 + half*128 + (h'%4)*32 + i      (h' = head within fc)


def _qk_perm():
    """perm[j_new] = j_orig for one 512-wide fc chunk."""
    p = np.zeros(512, dtype=np.int64)
    for hp in range(8):
        hi, hlo = hp // 4, hp % 4
        for half in range(2):
            for i in range(32):
                jn = hi * 256 + half * 128 + hlo * 32 + i
                p[jn] = hp * 64 + half * 32 + i
    return p


_QKPERM = _qk_perm()
_QKPERM2 = np.concatenate([_QKPERM, 512 + _QKPERM])  # both fc chunks


def _pack_kpairs(w):
    """[1024, N] -> [128, 4, 2, N] with rows d = (2j+t)*128 + p."""
    n = w.shape[1]
    return np.ascontiguousarray(
        w.reshape(4, 2, 128, n).transpose(2, 0, 1, 3)
    )


def _f8(a):
    return np.ascontiguousarray(np.asarray(a, dtype=np.float32)).astype(NP_FP8)


def _bf(a):
    return np.ascontiguousarray(np.asarray(a, dtype=np.float32)).astype(NP_BF16)


def _make_in_maps(x, c, norm1_w, norm2_w, w_qkv, w_out, w1, b1, w2, b2,
                  adaLN_w, adaLN_b, cos, sin):
    f32 = lambda a: np.asarray(a, dtype=np.float32)
    x = f32(x); c = f32(c)
    w_qkv = f32(w_qkv); w_out = f32(w_out); w1 = f32(w1); w2 = f32(w2)
    cos = f32(cos); sin = f32(sin)

    # adaLN modulation on host: [B, 6D]
    mod = c @ f32(adaLN_w) + f32(adaLN_b)
    sh_msa, sc_msa, g_msa, sh_mlp, sc_mlp, g_mlp = np.split(mod, 6, axis=-1)

    in_maps = []
    for core in range(N_CORES):
        b, half = core // 2, core % 2
        sh = -half * SQ

        m1 = (1.0 + sc_msa[b]) * f32(norm1_w)      # [D]
        m2 = (1.0 + sc_mlp[b]) * f32(norm2_w)
        wqkv_s = w_qkv * m1[:, None]               # [D, 3D]
        biasqkv = sh_msa[b] @ w_qkv                # [3D]

        wq_c = wqkv_s[:, 0:D][:, _QKPERM2]
        wk_c = wqkv_s[:, D : 2 * D][:, _QKPERM2]
        wv_c = wqkv_s[:, 2 * D : 3 * D]
        bq_c = biasqkv[0:D][_QKPERM2]
        bk_c = biasqkv[D : 2 * D][_QKPERM2]
        bv_c = biasqkv[2 * D : 3 * D]

        # wout: columns gated; rows follow the attnT8 partition layout
        wout_s = w_out * g_msa[b][None, :]
        dmap = np.zeros(D, dtype=np.int64)
        for p in range(128):
            for s in range(8):
                dmap[s * 128 + p] = (2 * s + p // 64) * 64 + (p % 64)
        wout_p = wout_s[dmap, :]

        w1_s = w1 * m2[:, None]
        bias1 = sh_mlp[b] @ w1 + f32(b1)
        w2_s = w2 * g_mlp[b][None, :]
        b2g = f32(b2) * g_mlp[b]

        sqkv = _pow2_scale(wqkv_s)
        swo = _pow2_scale(wout_p)
        sw1 = _pow2_scale(w1_s)
        sw2 = _pow2_scale(w2_s)
        sclv = np.zeros(8, dtype=np.float32)
        sclv[0], sclv[1], sclv[2], sclv[3] = 1/sqkv, 1/swo, 1/sw1, 1/sw2

        # rope tables, rotated with the tokens: row p -> freq index p%32
        idx = (np.arange(S) + half * SQ) % S
        cosr = cos[idx][:, None, :]                # [S, 1, 32]
        cosT = np.tile(cosr.transpose(2, 1, 0).reshape(32, S), (4, 1))
        sinr = sin[idx][:, None, :]
        sinT = np.tile(sinr.transpose(2, 1, 0).reshape(32, S), (4, 1))

        in_maps.append(dict(
            xb=_bf(np.roll(x[b], sh, axis=0)),
            wq=_f8(_pack_kpairs(wq_c * sqkv)),
            wk=_f8(_pack_kpairs(wk_c * sqkv)),
            wv=_f8(_pack_kpairs(wv_c * sqkv)),
            wo=_f8(_pack_kpairs(wout_p * swo)),
            w1=_pack_2lvl(w1_s, sw1),
            w2=_pack_2lvl(w2_s, sw2),
            scl=np.ascontiguousarray(np.tile(sclv[None, :], (128, 1))),
            cosT=_bf(cosT), sinT=_bf(sinT),
            bq=_bf(np.tile(bq_c[None, :], (128, 1))),
            bk=_bf(np.tile(bk_c[None, :], (128, 1))),
            bv=_bf(np.tile(bv_c[None, :], (128, 1))),
            b2r=_bf(np.tile(b2g[None, :], (128, 1))),
            b1r=np.ascontiguousarray(bias1.reshape(32, 128).T.astype(np.float32)),
        ))
    return in_maps


def _gather(results, x_shape):
    B = x_shape[0]
    out = np.empty(x_shape, dtype=np.float32)
    for core in range(N_CORES):
        b, half = core // 2, core % 2
        out[b, half * SQ : (half + 1) * SQ] = results[core]["out"]
    return out


def run(inputs, trace=False, reps=1):
    nc = _get_nc(reps)
    in_maps = _make_in_maps(**inputs)
    res = run_bass_kernel_spmd(nc, in_maps, list(range(N_CORES)), trace=trace)
    out = _gather(res.results, np.asarray(inputs["x"]).shape)
    return out, res


def kernel(**inputs):
    out, _ = run(inputs)
    return out
